# revision 21
# baseline (speedup 1.0000x reference)
"""Trainium2 Bass kernel for MemoryEfficientAttnBlock3D.

Computes: y = x + conv1x1(attn(conv1x1_{q,k,v}(groupnorm(x))), wp, bp)
for x of shape (2, 64, 32, 32, 8)  (B=2, C=64, N=8192 tokens per batch).

Sharding: 8 cores = 2 batches x 4 query-chunks of 2048 tokens.  Each core
receives its batch's full token volume ROTATED so that its query chunk is
always tokens [0:2048] -- groupnorm statistics and softmax/AV reductions are
permutation-invariant over kv tokens, so all cores run an identical program.

Algebraic folds done on the host (all exact in fp32 up to rounding):
  - gamma folds into Wq/Wk/Wv columns; the attention scale 1/sqrt(C) folds
    into Wq.
  - k's additive constant (Wk@beta + bk) shifts every score of a softmax row
    equally -> dropped exactly.
  - bq enters scores via an extra contraction row: q gets a constant ones
    row, k gets an extra output row (Wk_eff^T @ bq_eff) @ xn.
  - v's additive constant bv_eff = wv@beta + bv is pulled through the
    attention average (softmax rows sum to 1) and folded into bp.
  - softmax denominators: v^T carries a ones column, so the AV matmul
    accumulates [AV | rowsum]; the division is applied after the output
    projection (column scaling commutes with the left-matmul), with
    bp entering as bp * rowsum / rowsum via row 64 of the wp matmul.
"""

import numpy as np

import concourse.bass as bass
import concourse.tile as tile
from concourse import bacc, mybir

F32 = mybir.dt.float32
F16 = mybir.dt.float16
AF = mybir.ActivationFunctionType
OP = mybir.AluOpType

# Attention matmul dtype. fp32 matmuls run 4x slower on the PE (LOW_HIGH
# weight split + half-rate fp32 streaming), so scores/AV use fp16 operands
# with fp32 PSUM accumulation: measured end-to-end absmax 3.1e-4 on outputs
# with |out|max 5.3 (fp32 floor is 1.5e-4... see notes) — ~6e-5 of scale.
ATTN_DT = F16

C = 64
GROUPS = 32
EPS = 1e-6

B_FULL = 2
H_FULL, W_FULL, D_FULL = 32, 32, 8
N_FULL = H_FULL * W_FULL * D_FULL  # 8192 kv tokens per batch
N_CORES = 8
Q_CHUNKS = 4
M_FULL = N_FULL // Q_CHUNKS  # 2048 q tokens per core

MB = 512        # q-token block (one PSUM bank of fp32)
NT = 128        # kv-token tile (matmul M / partition dim)
GSZ = 3         # n-tiles per exp group ([128, 1536] PSUM tile = 3 banks)
STAT_CHUNK = 1024
WARMUP_MMS = 56  # dummy matmuls to lift the PE HAM clock-gate to 8/8 early


def emit(tc, nc, n_tok, m_tok, xb_d, wq_d, wk_d, wv_d, wp_d, pair_d, expand_d, out_d):
    ntiles = n_tok // NT
    nch = max(1, n_tok // STAT_CHUNK)
    sch = n_tok // nch
    pch = 512  # projection chunk

    with (
        tc.tile_pool(name="persist", bufs=1) as persist,
        tc.tile_pool(name="expS", bufs=4) as epool,
        tc.tile_pool(name="mtail", bufs=3) as mpool,
        tc.tile_pool(name="spsum", bufs=2, space="PSUM") as spool,
        tc.tile_pool(name="apsum", bufs=2, space="PSUM") as apool,
        tc.tile_pool(name="dram", bufs=2, space="DRAM") as dpool,
    ):
        # ---- persistent SBUF tensors ----
        xb_sb = persist.tile([C, n_tok], F32)
        xn_sb = persist.tile([C, n_tok], F32)
        k_sb = persist.tile([C + 1, n_tok], ATTN_DT)
        q_sb = persist.tile([C + 1, m_tok], ATTN_DT)
        vt_sb = persist.tile([NT, ntiles * (C + 1)], ATTN_DT)
        wq_sb = persist.tile([C, C], F32)
        wk_sb = persist.tile([C, C + 1], F32)
        wv_sb = persist.tile([C, C], F32)
        wp_sb = persist.tile([C + 1, C], F32)
        pair_sb = persist.tile([C, GROUPS], F32)
        expand_sb = persist.tile([GROUPS, C], F32)
        stats_sb = persist.tile([C, 2 * nch], F32)
        scratch_sb = persist.tile([C, sch], F32)
        eps_sb = persist.tile([GROUPS, 1], F32)
        mrg_sb = persist.tile([GROUPS, 2], F32)
        mrc_sb = persist.tile([C, 2], F32)

        nc.sync.dma_start(out=wq_sb[:], in_=wq_d[:, :])
        nc.sync.dma_start(out=wk_sb[:], in_=wk_d[:, :])
        nc.sync.dma_start(out=wv_sb[:], in_=wv_d[:, :])
        nc.sync.dma_start(out=wp_sb[:], in_=wp_d[:, :])
        nc.sync.dma_start(out=pair_sb[:], in_=pair_d[:, :])
        nc.sync.dma_start(out=expand_sb[:], in_=expand_d[:, :])
        nc.vector.memset(eps_sb[:], EPS)
        # ones column (col C of each 65-wide v^T block) -> AV rowsum; ones row
        # of q -> bq contribution to scores.
        nc.gpsimd.memset(vt_sb[:], 1.0)
        nc.gpsimd.memset(q_sb[C : C + 1, :], 1.0)

        # PE warm-up: the HAM clock gate keeps the PE at 1.2 GHz until it has
        # seen ~3.4us of sustained matmul activity.  Burn dummy matmuls (the
        # PE is otherwise idle while DMA + stats run) so the prologue
        # projections and attention start at 2.4 GHz.
        junk_sb = persist.tile([C, 256], F32)
        nc.gpsimd.memset(junk_sb[:], 0.0)
        junk_ps = spool.tile([C, 256], F32, tag="s")
        for _ in range(WARMUP_MMS):
            nc.tensor.matmul(
                junk_ps[:], junk_sb[:, 0:C], junk_sb[:], start=True, stop=True
            )
        # consume the result so dead-code elimination keeps the warm-up chain
        nc.vector.tensor_copy(junk_sb[0:1, 0:1], junk_ps[0:1, 0:1])

        # ---- load x and accumulate per-channel sum / sum-of-squares ----
        for ch in range(nch):
            sl = slice(ch * sch, (ch + 1) * sch)
            nc.sync.dma_start(out=xb_sb[:, sl], in_=xb_d[:, sl])
            # sum into stats col ch (xn_sb doubles as a scratch dump)
            nc.vector.tensor_scalar(
                out=xn_sb[:, sl], in0=xb_sb[:, sl], scalar1=1.0, scalar2=None,
                op0=OP.mult, op1=OP.add, accum_out=stats_sb[:, ch : ch + 1],
            )
            # sum of squares into stats col nch+ch (scratch as dump)
            nc.scalar.activation(
                out=scratch_sb[:], in_=xb_sb[:, sl], func=AF.Square,
                accum_out=stats_sb[:, nch + ch : nch + ch + 1],
            )

        # ---- group statistics: pair-sum across channel pairs + chunks ----
        gp = apool.tile([GROUPS, 2 * nch], F32, tag="ap")
        nc.tensor.matmul(gp[:], pair_sb[:], stats_sb[:], start=True, stop=True)
        # [32, 2*nch] -> [32, 2, nch] -> reduce innermost -> [32, 2] = [mean, Ex2]
        gsum = mpool.tile([GROUPS, 2], F32, tag="gsum")
        nc.vector.tensor_reduce(
            out=gsum[:], in_=gp[:].rearrange("p (s c) -> p s c", s=2),
            axis=mybir.AxisListType.X, op=OP.add,
        )
        # var = Ex2 - mean^2 ; rstd = 1/sqrt(var + eps) ; keep [mean, rstd]
        msq = mpool.tile([GROUPS, 1], F32, tag="msq")
        nc.vector.tensor_mul(msq[:], gsum[:, 0:1], gsum[:, 0:1])
        nc.vector.tensor_copy(mrg_sb[:, 0:1], gsum[:, 0:1])
        nc.vector.tensor_sub(mrg_sb[:, 1:2], gsum[:, 1:2], msq[:])
        nc.scalar.activation(
            out=mrg_sb[:, 1:2], in_=mrg_sb[:, 1:2], func=AF.Sqrt, bias=eps_sb[:],
        )
        nc.vector.reciprocal(mrg_sb[:, 1:2], mrg_sb[:, 1:2])
        # expand group stats back to per-channel [64, 2]
        ep = apool.tile([C, 2], F32, tag="ap")
        nc.tensor.matmul(ep[:], expand_sb[:], mrg_sb[:], start=True, stop=True)
        nc.vector.tensor_copy(mrc_sb[:], ep[:])

        # ---- normalize + projections, interleaved so attention can start
        # as soon as the first k/v tiles exist.  q projections come first
        # (they cover tokens [0, m_tok) = the first xn chunks).
        vt_view = vt_sb[:].rearrange("p (t e) -> p t e", e=C + 1)
        xch = max(1, n_tok // 2048)   # xn/projection macro-chunks
        xsz = n_tok // xch

        def proj_k(j):
            sl = slice(j * pch, (j + 1) * pch)
            kp = apool.tile([C + 1, pch], F32, tag="ap")
            nc.tensor.matmul(kp[:], wk_sb[:], xn_sb[:, sl], start=True, stop=True)
            nc.vector.tensor_copy(k_sb[:, sl], kp[:])

        def proj_vt(j4):
            t0, tn = j4 * 4, min(4, ntiles - j4 * 4)
            vp = apool.tile([NT, tn * C], F32, tag="ap")
            for t in range(tn):
                j = t0 + t
                nc.tensor.matmul(
                    vp[:, t * C : (t + 1) * C],
                    xn_sb[:, j * NT : (j + 1) * NT], wv_sb[:],
                    start=True, stop=True,
                )
            nc.vector.tensor_copy(
                vt_view[:, t0 : t0 + tn, 0:C],
                vp[:].rearrange("p (t e) -> p t e", e=C),
            )

        for ch in range(xch):
            sl = slice(ch * xsz, (ch + 1) * xsz)
            nc.vector.tensor_scalar(
                out=xn_sb[:, sl], in0=xb_sb[:, sl],
                scalar1=mrc_sb[:, 0:1], scalar2=mrc_sb[:, 1:2],
                op0=OP.subtract, op1=OP.mult,
            )
            if ch == 0:
                for j in range(m_tok // pch):
                    qsl = slice(j * pch, (j + 1) * pch)
                    qp = apool.tile([C, pch], F32, tag="ap")
                    nc.tensor.matmul(
                        qp[:], wq_sb[:], xn_sb[:, qsl], start=True, stop=True
                    )
                    nc.vector.tensor_copy(q_sb[0:C, qsl], qp[:])
            for j in range(ch * (xsz // pch), (ch + 1) * (xsz // pch)):
                proj_k(j)
                proj_vt(j)  # pch == 4*NT: k-chunk j covers v^T tile group j

        # ---- attention, one 512-query block at a time; the previous
        # block's tail (normalize/project/residual) is emitted a few groups
        # into the next block so its fp32 matmuls and DVE work fill engine
        # gaps instead of stalling the PE/ACT pipeline at the boundary ----
        def make_tail(av, msl):
            def tail():
                av_sb = mpool.tile([C + 1, MB], F32, tag="avsb", name="av_sb")
                nc.vector.tensor_copy(av_sb[:], av[:])
                recip = mpool.tile([1, MB], F32, tag="recip", name="recip")
                rscr = mpool.tile([1, MB], F32, tag="rscr", name="rscr")
                nc.vector.reciprocal_approx_accurate(
                    recip[:], av_sb[C : C + 1, :], rscr[:]
                )
                # partition-broadcast recip via a DRAM bounce (SBUF-source
                # DMA cannot replicate across partitions)
                rd = dpool.tile([1, MB], F32, tag="rd", name="rd")
                nc.sync.dma_start(out=rd[:], in_=recip[:])
                rb = mpool.tile([C, MB], F32, tag="rb", name="rb")
                nc.sync.dma_start(out=rb[:], in_=rd[:].to_broadcast([C, MB]))
                pp = apool.tile([C, MB], F32, tag="ap", name="pp")
                nc.tensor.matmul(pp[:], wp_sb[:], av_sb[:], start=True, stop=True)
                t1 = mpool.tile([C, MB], F32, tag="t1", name="t1")
                nc.vector.tensor_mul(t1[:], pp[:], rb[:])
                outt = mpool.tile([C, MB], F32, tag="outt", name="outt")
                nc.vector.tensor_add(outt[:], t1[:], xb_sb[:, msl])
                nc.sync.dma_start(out=out_d[:, msl], in_=outt[:])
            return tail

        deferred = None
        for mb in range(m_tok // MB):
            msl = slice(mb * MB, (mb + 1) * MB)
            av = apool.tile([C + 1, MB], F32, tag="ap")
            pending = None
            for gi, g0 in enumerate(range(0, ntiles, GSZ)):
                gsz = min(GSZ, ntiles - g0)
                sp = spool.tile([NT, gsz * MB], F32, tag="s")
                for t in range(gsz):
                    j = g0 + t
                    nc.tensor.matmul(
                        sp[:, t * MB : (t + 1) * MB],
                        k_sb[:, j * NT : (j + 1) * NT], q_sb[:, msl],
                        start=True, stop=True,
                    )
                ex = epool.tile([NT, gsz * MB], ATTN_DT, tag="e")
                nc.scalar.activation(out=ex[:], in_=sp[:], func=AF.Exp)
                if pending is not None:
                    pg0, psz, pex = pending
                    for t in range(psz):
                        j = pg0 + t
                        nc.tensor.matmul(
                            av[:], vt_view[:, j, :], pex[:, t * MB : (t + 1) * MB],
                            start=(j == 0), stop=(j == ntiles - 1),
                        )
                pending = (g0, gsz, ex)
                if gi == 3 and deferred is not None:
                    deferred()
                    deferred = None
            pg0, psz, pex = pending
            for t in range(psz):
                j = pg0 + t
                nc.tensor.matmul(
                    av[:], vt_view[:, j, :], pex[:, t * MB : (t + 1) * MB],
                    start=(j == 0), stop=(j == ntiles - 1),
                )
            if deferred is not None:  # few-group case: gi==3 never fired
                deferred()
            deferred = make_tail(av, msl)
        deferred()


def build_program(n_tok=N_FULL, m_tok=M_FULL):
    nc = bacc.Bacc("TRN2", target_bir_lowering=False, debug=False)
    xb_d = nc.dram_tensor("xb", [C, n_tok], F32, kind="ExternalInput")
    wq_d = nc.dram_tensor("wqT", [C, C], F32, kind="ExternalInput")
    wk_d = nc.dram_tensor("wkT", [C, C + 1], F32, kind="ExternalInput")
    wv_d = nc.dram_tensor("wvT", [C, C], F32, kind="ExternalInput")
    wp_d = nc.dram_tensor("wpT", [C + 1, C], F32, kind="ExternalInput")
    pair_d = nc.dram_tensor("pair", [C, GROUPS], F32, kind="ExternalInput")
    expand_d = nc.dram_tensor("expand", [GROUPS, C], F32, kind="ExternalInput")
    out_d = nc.dram_tensor("out", [C, m_tok], F32, kind="ExternalOutput")
    with tile.TileContext(nc) as tc:
        emit(tc, nc, n_tok, m_tok,
             xb_d.ap(), wq_d.ap(), wk_d.ap(), wv_d.ap(), wp_d.ap(),
             pair_d.ap(), expand_d.ap(), out_d.ap())
    nc.compile()
    return nc


def prep_weights(gamma, beta, wq, bq, wk, bk, wv, bv, wp, bp, n_tok=N_FULL):
    """Host-side algebraic folds. Returns the shared per-core input dict."""
    f32 = np.float32
    gamma, beta = gamma.astype(f32), beta.astype(f32)
    scale = f32(1.0) / np.sqrt(f32(C)).astype(f32)
    wq_eff = (wq * gamma[None, :]) * scale
    bq_eff = (wq @ beta + bq) * scale
    wk_eff = wk * gamma[None, :]
    wv_eff = wv * gamma[None, :]
    bv_eff = wv @ beta + bv
    bp_eff = bp + wp @ bv_eff

    wkT = np.zeros((C, C + 1), f32)
    wkT[:, 0:C] = wk_eff.T
    wkT[:, C] = wk_eff.T @ bq_eff
    wpT = np.zeros((C + 1, C), f32)
    wpT[0:C, :] = wp.T
    wpT[C, :] = bp_eff
    pair = np.zeros((C, GROUPS), f32)
    pair[np.arange(C), np.arange(C) // 2] = f32(1.0) / f32(2 * n_tok)
    expand = np.zeros((GROUPS, C), f32)
    expand[np.arange(C) // 2, np.arange(C)] = 1.0
    return {
        "wqT": np.ascontiguousarray(wq_eff.T, f32),
        "wkT": np.ascontiguousarray(wkT, f32),
        "wvT": np.ascontiguousarray(wv_eff.T, f32),
        "wpT": np.ascontiguousarray(wpT, f32),
        "pair": pair,
        "expand": expand,
    }


_PROGRAM_CACHE = {}


def _get_program(n_tok, m_tok):
    key = (n_tok, m_tok)
    if key not in _PROGRAM_CACHE:
        _PROGRAM_CACHE[key] = build_program(n_tok, m_tok)
    return _PROGRAM_CACHE[key]


def make_in_maps(x, shared):
    """Per-core input maps: batch b = core//4, query chunk qc = core%4."""
    in_maps = []
    for core in range(N_CORES):
        b, qc = core // Q_CHUNKS, core % Q_CHUNKS
        xb = np.ascontiguousarray(x[b].reshape(C, N_FULL), np.float32)
        xb = np.ascontiguousarray(np.roll(xb, -qc * M_FULL, axis=1))
        in_maps.append({"xb": xb, **shared})
    return in_maps


def kernel(x, gamma, beta, wq, bq, wk, bk, wv, bv, wp, bp, **run_kwargs):
    from concourse.bass_utils import run_bass_kernel_spmd

    x = np.asarray(x, np.float32)
    shared = prep_weights(
        np.asarray(gamma), np.asarray(beta), np.asarray(wq), np.asarray(bq),
        np.asarray(wk), np.asarray(bk), np.asarray(wv), np.asarray(bv),
        np.asarray(wp), np.asarray(bp),
    )
    nc = _get_program(N_FULL, M_FULL)
    in_maps = make_in_maps(x, shared)
    res = run_bass_kernel_spmd(nc, in_maps, core_ids=list(range(N_CORES)), **run_kwargs)
    y = np.empty((B_FULL, C, N_FULL), np.float32)
    for core in range(N_CORES):
        b, qc = core // Q_CHUNKS, core % Q_CHUNKS
        y[b, :, qc * M_FULL : (qc + 1) * M_FULL] = res.results[core]["out"]
    out = y.reshape(B_FULL, C, H_FULL, W_FULL, D_FULL)
    if run_kwargs:
        return out, res
    return out


# revision 25
# speedup vs baseline: 1.2792x; 1.2792x over previous
"""Trainium2 Bass kernel for MemoryEfficientAttnBlock3D.

Computes: y = x + conv1x1(attn(conv1x1_{q,k,v}(groupnorm(x))), wp, bp)
for x of shape (2, 64, 32, 32, 8)  (B=2, C=64, N=8192 tokens per batch).

Sharding: 8 cores = 2 batches x 4 query-chunks of 2048 tokens.  Each core
receives its batch's full token volume ROTATED so that its query chunk is
always tokens [0:2048] -- groupnorm statistics and softmax/AV reductions are
permutation-invariant over kv tokens, so all cores run an identical program.

Algebraic folds done on the host (all exact in fp32 up to rounding):
  - gamma folds into Wq/Wk/Wv columns; the attention scale 1/sqrt(C) folds
    into Wq.
  - k's additive constant (Wk@beta + bk) shifts every score of a softmax row
    equally -> dropped exactly.
  - bq enters scores via an extra contraction row: q gets a constant ones
    row, k gets an extra output row (Wk_eff^T @ bq_eff) @ xn.
  - v's additive constant bv_eff = wv@beta + bv is pulled through the
    attention average (softmax rows sum to 1) and folded into bp.
  - softmax denominators: v^T carries a ones column, so the AV matmul
    accumulates [AV | rowsum]; the division is applied after the output
    projection (column scaling commutes with the left-matmul), with
    bp entering as bp * rowsum / rowsum via row 64 of the wp matmul.
"""

import numpy as np

import concourse.bass as bass
import concourse.tile as tile
from concourse import bacc, mybir

F32 = mybir.dt.float32
F16 = mybir.dt.float16
AF = mybir.ActivationFunctionType
OP = mybir.AluOpType

# Attention matmul dtype. fp32 matmuls run 4x slower on the PE (LOW_HIGH
# weight split + half-rate fp32 streaming), so scores/AV use fp16 operands
# with fp32 PSUM accumulation: measured end-to-end absmax 3.1e-4 on outputs
# with |out|max 5.3 (fp32 floor is 1.5e-4... see notes) — ~6e-5 of scale.
ATTN_DT = F16

C = 64
GROUPS = 32
EPS = 1e-6

B_FULL = 2
H_FULL, W_FULL, D_FULL = 32, 32, 8
N_FULL = H_FULL * W_FULL * D_FULL  # 8192 kv tokens per batch
N_CORES = 8
Q_CHUNKS = 4
M_FULL = N_FULL // Q_CHUNKS  # 2048 q tokens per core

MB = 512        # q-token block (one PSUM bank of fp32)
NT = 128        # kv-token tile (matmul M / partition dim)
GSZ = 3         # n-tiles per exp group ([128, 1536] PSUM tile = 3 banks)
STAT_CHUNK = 1024


def emit(tc, nc, n_tok, m_tok, xb_d, wqh_d, wql_d, wkh_d, wkl_d, wvh_d, wvl_d,
         wp_d, pair_d, expand_d, out_d):
    ntiles = n_tok // NT
    nch = max(1, n_tok // STAT_CHUNK)
    sch = n_tok // nch
    pch = 512  # projection chunk

    with (
        tc.tile_pool(name="persist", bufs=1) as persist,
        tc.tile_pool(name="expS", bufs=4) as epool,
        tc.tile_pool(name="mtail", bufs=3) as mpool,
        tc.tile_pool(name="spsum", bufs=2, space="PSUM") as spool,
        tc.tile_pool(name="apsum", bufs=2, space="PSUM") as apool,
        tc.tile_pool(name="dram", bufs=2, space="DRAM") as dpool,
    ):
        # ---- persistent SBUF tensors ----
        xb_sb = persist.tile([C, n_tok], F32)
        xh_sb = persist.tile([C, n_tok], ATTN_DT)
        k_sb = persist.tile([C + 1, n_tok], ATTN_DT)
        q_sb = persist.tile([C + 1, m_tok], ATTN_DT)
        vt_sb = persist.tile([NT, ntiles * (C + 1)], ATTN_DT)
        wqh_sb = persist.tile([C, C], ATTN_DT)
        wql_sb = persist.tile([C, C], ATTN_DT)
        wkh_sb = persist.tile([C, C + 1], ATTN_DT)
        wkl_sb = persist.tile([C, C + 1], ATTN_DT)
        wvh_sb = persist.tile([C, C], ATTN_DT)
        wvl_sb = persist.tile([C, C], ATTN_DT)
        wp_sb = persist.tile([C + 1, C], F32)
        pair_sb = persist.tile([C, GROUPS], F32)
        expand_sb = persist.tile([GROUPS, C], F32)
        stats_sb = persist.tile([C, 2 * nch], F32)
        scratch_sb = persist.tile([C, sch], F32)
        scratch2_sb = persist.tile([C, sch], F32)
        eps_sb = persist.tile([GROUPS, 1], F32)
        mrg_sb = persist.tile([GROUPS, 2], F32)
        mrc_sb = persist.tile([C, 2], F32)

        nc.sync.dma_start(out=wqh_sb[:], in_=wqh_d[:, :])
        nc.sync.dma_start(out=wql_sb[:], in_=wql_d[:, :])
        nc.sync.dma_start(out=wkh_sb[:], in_=wkh_d[:, :])
        nc.sync.dma_start(out=wkl_sb[:], in_=wkl_d[:, :])
        nc.sync.dma_start(out=wvh_sb[:], in_=wvh_d[:, :])
        nc.sync.dma_start(out=wvl_sb[:], in_=wvl_d[:, :])
        nc.sync.dma_start(out=wp_sb[:], in_=wp_d[:, :])
        nc.sync.dma_start(out=pair_sb[:], in_=pair_d[:, :])
        nc.sync.dma_start(out=expand_sb[:], in_=expand_d[:, :])
        nc.vector.memset(eps_sb[:], EPS)
        # ones column (col C of each 65-wide v^T block) -> AV rowsum; ones row
        # of q -> bq contribution to scores.
        nc.gpsimd.memset(vt_sb[:], 1.0)
        nc.gpsimd.memset(q_sb[C : C + 1, :], 1.0)


        # ---- load x and accumulate per-channel sum / sum-of-squares ----
        for ch in range(nch):
            sl = slice(ch * sch, (ch + 1) * sch)
            nc.sync.dma_start(out=xb_sb[:, sl], in_=xb_d[:, sl])
            # sum into stats col ch (xn_sb doubles as a scratch dump)
            nc.vector.tensor_scalar(
                out=scratch2_sb[:], in0=xb_sb[:, sl], scalar1=1.0, scalar2=None,
                op0=OP.mult, op1=OP.add, accum_out=stats_sb[:, ch : ch + 1],
            )
            # sum of squares into stats col nch+ch (scratch as dump)
            nc.scalar.activation(
                out=scratch_sb[:], in_=xb_sb[:, sl], func=AF.Square,
                accum_out=stats_sb[:, nch + ch : nch + ch + 1],
            )

        # ---- group statistics: pair-sum across channel pairs + chunks ----
        gp = apool.tile([GROUPS, 2 * nch], F32, tag="ap")
        nc.tensor.matmul(gp[:], pair_sb[:], stats_sb[:], start=True, stop=True)
        # [32, 2*nch] -> [32, 2, nch] -> reduce innermost -> [32, 2] = [mean, Ex2]
        gsum = mpool.tile([GROUPS, 2], F32, tag="gsum")
        nc.vector.tensor_reduce(
            out=gsum[:], in_=gp[:].rearrange("p (s c) -> p s c", s=2),
            axis=mybir.AxisListType.X, op=OP.add,
        )
        # var = Ex2 - mean^2 ; rstd = 1/sqrt(var + eps) ; keep [mean, rstd]
        msq = mpool.tile([GROUPS, 1], F32, tag="msq")
        nc.vector.tensor_mul(msq[:], gsum[:, 0:1], gsum[:, 0:1])
        nc.vector.tensor_copy(mrg_sb[:, 0:1], gsum[:, 0:1])
        nc.vector.tensor_sub(mrg_sb[:, 1:2], gsum[:, 1:2], msq[:])
        nc.scalar.activation(
            out=mrg_sb[:, 1:2], in_=mrg_sb[:, 1:2], func=AF.Sqrt, bias=eps_sb[:],
        )
        nc.vector.reciprocal(mrg_sb[:, 1:2], mrg_sb[:, 1:2])
        # expand group stats back to per-channel [64, 2]
        ep = apool.tile([C, 2], F32, tag="ap")
        nc.tensor.matmul(ep[:], expand_sb[:], mrg_sb[:], start=True, stop=True)
        nc.vector.tensor_copy(mrc_sb[:], ep[:])

        # ---- normalize + projections, interleaved so attention can start
        # as soon as the first k/v tiles exist.  q projections come first
        # (they cover tokens [0, m_tok) = the first xn chunks).
        vt_view = vt_sb[:].rearrange("p (t e) -> p t e", e=C + 1)
        xch = max(1, n_tok // 2048)   # xn/projection macro-chunks
        xsz = n_tok // xch

        def proj_k(j):
            sl = slice(j * pch, (j + 1) * pch)
            kp = apool.tile([C + 1, pch], F32, tag="ap")
            nc.tensor.matmul(kp[:], wkh_sb[:], xh_sb[:, sl], start=True, stop=False)
            nc.tensor.matmul(kp[:], wkl_sb[:], xh_sb[:, sl], start=False, stop=True)
            nc.vector.tensor_copy(k_sb[:, sl], kp[:])

        def proj_vt(j4):
            t0, tn = j4 * 4, min(4, ntiles - j4 * 4)
            vp = apool.tile([NT, tn * C], F32, tag="ap")
            for t in range(tn):
                j = t0 + t
                xh_t = xh_sb[:, j * NT : (j + 1) * NT]
                nc.tensor.matmul(
                    vp[:, t * C : (t + 1) * C], xh_t, wvh_sb[:],
                    start=True, stop=False,
                )
                nc.tensor.matmul(
                    vp[:, t * C : (t + 1) * C], xh_t, wvl_sb[:],
                    start=False, stop=True,
                )
            nc.vector.tensor_copy(
                vt_view[:, t0 : t0 + tn, 0:C],
                vp[:].rearrange("p (t e) -> p t e", e=C),
            )

        for ch in range(xch):
            sl = slice(ch * xsz, (ch + 1) * xsz)
            nc.vector.tensor_scalar(
                out=xh_sb[:, sl], in0=xb_sb[:, sl],
                scalar1=mrc_sb[:, 0:1], scalar2=mrc_sb[:, 1:2],
                op0=OP.subtract, op1=OP.mult,
            )
            if ch == 0:
                for j in range(m_tok // pch):
                    qsl = slice(j * pch, (j + 1) * pch)
                    qp = apool.tile([C, pch], F32, tag="ap")
                    nc.tensor.matmul(
                        qp[:], wqh_sb[:], xh_sb[:, qsl], start=True, stop=False
                    )
                    nc.tensor.matmul(
                        qp[:], wql_sb[:], xh_sb[:, qsl], start=False, stop=True
                    )
                    nc.vector.tensor_copy(q_sb[0:C, qsl], qp[:])
            for j in range(ch * (xsz // pch), (ch + 1) * (xsz // pch)):
                proj_k(j)
                proj_vt(j)  # pch == 4*NT: k-chunk j covers v^T tile group j

        # ---- attention, one 512-query block at a time; the previous
        # block's tail (normalize/project/residual) is emitted a few groups
        # into the next block so its fp32 matmuls and DVE work fill engine
        # gaps instead of stalling the PE/ACT pipeline at the boundary ----
        def make_tail(av, msl):
            def tail():
                av_sb = mpool.tile([C + 1, MB], F32, tag="avsb", name="av_sb")
                nc.vector.tensor_copy(av_sb[:], av[:])
                recip = mpool.tile([1, MB], F32, tag="recip", name="recip")
                nc.vector.reciprocal(recip[:], av_sb[C : C + 1, :])
                # partition-broadcast recip via a DRAM bounce (SBUF-source
                # DMA cannot replicate across partitions)
                rd = dpool.tile([1, MB], F32, tag="rd", name="rd")
                nc.sync.dma_start(out=rd[:], in_=recip[:])
                rb = mpool.tile([C, MB], F32, tag="rb", name="rb")
                nc.sync.dma_start(out=rb[:], in_=rd[:].to_broadcast([C, MB]))
                pp = apool.tile([C, MB], F32, tag="ap", name="pp")
                nc.tensor.matmul(pp[:], wp_sb[:], av_sb[:], start=True, stop=True)
                t1 = mpool.tile([C, MB], F32, tag="t1", name="t1")
                nc.vector.tensor_mul(t1[:], pp[:], rb[:])
                outt = mpool.tile([C, MB], F32, tag="outt", name="outt")
                nc.vector.tensor_add(outt[:], t1[:], xb_sb[:, msl])
                nc.sync.dma_start(out=out_d[:, msl], in_=outt[:])
            return tail

        deferred = None
        for mb in range(m_tok // MB):
            msl = slice(mb * MB, (mb + 1) * MB)
            av = apool.tile([C + 1, MB], F32, tag="ap")
            pending = None
            for gi, g0 in enumerate(range(0, ntiles, GSZ)):
                gsz = min(GSZ, ntiles - g0)
                sp = spool.tile([NT, gsz * MB], F32, tag="s")
                for t in range(gsz):
                    j = g0 + t
                    nc.tensor.matmul(
                        sp[:, t * MB : (t + 1) * MB],
                        k_sb[:, j * NT : (j + 1) * NT], q_sb[:, msl],
                        start=True, stop=True,
                    )
                ex = epool.tile([NT, gsz * MB], ATTN_DT, tag="e")
                nc.scalar.activation(out=ex[:], in_=sp[:], func=AF.Exp)
                if pending is not None:
                    pg0, psz, pex = pending
                    for t in range(psz):
                        j = pg0 + t
                        nc.tensor.matmul(
                            av[:], vt_view[:, j, :], pex[:, t * MB : (t + 1) * MB],
                            start=(j == 0), stop=(j == ntiles - 1),
                        )
                pending = (g0, gsz, ex)
                if gi == 3 and deferred is not None:
                    deferred()
                    deferred = None
            pg0, psz, pex = pending
            for t in range(psz):
                j = pg0 + t
                nc.tensor.matmul(
                    av[:], vt_view[:, j, :], pex[:, t * MB : (t + 1) * MB],
                    start=(j == 0), stop=(j == ntiles - 1),
                )
            if deferred is not None:  # few-group case: gi==3 never fired
                deferred()
            deferred = make_tail(av, msl)
        deferred()


def build_program(n_tok=N_FULL, m_tok=M_FULL):
    nc = bacc.Bacc("TRN2", target_bir_lowering=False, debug=False)
    xb_d = nc.dram_tensor("xb", [C, n_tok], F32, kind="ExternalInput")
    wqh_d = nc.dram_tensor("wqh", [C, C], F16, kind="ExternalInput")
    wql_d = nc.dram_tensor("wql", [C, C], F16, kind="ExternalInput")
    wkh_d = nc.dram_tensor("wkh", [C, C + 1], F16, kind="ExternalInput")
    wkl_d = nc.dram_tensor("wkl", [C, C + 1], F16, kind="ExternalInput")
    wvh_d = nc.dram_tensor("wvh", [C, C], F16, kind="ExternalInput")
    wvl_d = nc.dram_tensor("wvl", [C, C], F16, kind="ExternalInput")
    wp_d = nc.dram_tensor("wpT", [C + 1, C], F32, kind="ExternalInput")
    pair_d = nc.dram_tensor("pair", [C, GROUPS], F32, kind="ExternalInput")
    expand_d = nc.dram_tensor("expand", [GROUPS, C], F32, kind="ExternalInput")
    out_d = nc.dram_tensor("out", [C, m_tok], F32, kind="ExternalOutput")
    with tile.TileContext(nc) as tc:
        emit(tc, nc, n_tok, m_tok,
             xb_d.ap(), wqh_d.ap(), wql_d.ap(), wkh_d.ap(), wkl_d.ap(),
             wvh_d.ap(), wvl_d.ap(), wp_d.ap(),
             pair_d.ap(), expand_d.ap(), out_d.ap())
    nc.compile()
    return nc


def prep_weights(gamma, beta, wq, bq, wk, bk, wv, bv, wp, bp, n_tok=N_FULL):
    """Host-side algebraic folds. Returns the shared per-core input dict."""
    f32 = np.float32
    gamma, beta = gamma.astype(f32), beta.astype(f32)
    scale = f32(1.0) / np.sqrt(f32(C)).astype(f32)
    wq_eff = (wq * gamma[None, :]) * scale
    bq_eff = (wq @ beta + bq) * scale
    wk_eff = wk * gamma[None, :]
    wv_eff = wv * gamma[None, :]
    bv_eff = wv @ beta + bv
    bp_eff = bp + wp @ bv_eff

    wkT = np.zeros((C, C + 1), f32)
    wkT[:, 0:C] = wk_eff.T
    wkT[:, C] = wk_eff.T @ bq_eff
    wpT = np.zeros((C + 1, C), f32)
    wpT[0:C, :] = wp.T
    wpT[C, :] = bp_eff
    pair = np.zeros((C, GROUPS), f32)
    pair[np.arange(C), np.arange(C) // 2] = f32(1.0) / f32(2 * n_tok)
    expand = np.zeros((GROUPS, C), f32)
    expand[np.arange(C) // 2, np.arange(C)] = 1.0

    def split16(a):
        hi = a.astype(np.float16)
        lo = (a - hi.astype(f32)).astype(np.float16)
        return hi, lo

    wqh, wql = split16(np.ascontiguousarray(wq_eff.T, f32))
    wkh, wkl = split16(np.ascontiguousarray(wkT, f32))
    wvh, wvl = split16(np.ascontiguousarray(wv_eff.T, f32))
    return {
        "wqh": wqh, "wql": wql,
        "wkh": wkh, "wkl": wkl,
        "wvh": wvh, "wvl": wvl,
        "wpT": np.ascontiguousarray(wpT, f32),
        "pair": pair,
        "expand": expand,
    }


_PROGRAM_CACHE = {}


def _get_program(n_tok, m_tok):
    key = (n_tok, m_tok)
    if key not in _PROGRAM_CACHE:
        _PROGRAM_CACHE[key] = build_program(n_tok, m_tok)
    return _PROGRAM_CACHE[key]


def make_in_maps(x, shared):
    """Per-core input maps: batch b = core//4, query chunk qc = core%4."""
    in_maps = []
    for core in range(N_CORES):
        b, qc = core // Q_CHUNKS, core % Q_CHUNKS
        xb = np.ascontiguousarray(x[b].reshape(C, N_FULL), np.float32)
        xb = np.ascontiguousarray(np.roll(xb, -qc * M_FULL, axis=1))
        in_maps.append({"xb": xb, **shared})
    return in_maps


def kernel(x, gamma, beta, wq, bq, wk, bk, wv, bv, wp, bp, **run_kwargs):
    from concourse.bass_utils import run_bass_kernel_spmd

    x = np.asarray(x, np.float32)
    shared = prep_weights(
        np.asarray(gamma), np.asarray(beta), np.asarray(wq), np.asarray(bq),
        np.asarray(wk), np.asarray(bk), np.asarray(wv), np.asarray(bv),
        np.asarray(wp), np.asarray(bp),
    )
    nc = _get_program(N_FULL, M_FULL)
    in_maps = make_in_maps(x, shared)
    res = run_bass_kernel_spmd(nc, in_maps, core_ids=list(range(N_CORES)), **run_kwargs)
    y = np.empty((B_FULL, C, N_FULL), np.float32)
    for core in range(N_CORES):
        b, qc = core // Q_CHUNKS, core % Q_CHUNKS
        y[b, :, qc * M_FULL : (qc + 1) * M_FULL] = res.results[core]["out"]
    out = y.reshape(B_FULL, C, H_FULL, W_FULL, D_FULL)
    if run_kwargs:
        return out, res
    return out
